# revision 21
# baseline (speedup 1.0000x reference)
"""GroupedQueryAttention (B=1, T=2048, C=2048, H=16, KVH=4, D=128) on 8 trn2 cores.

Sharding: tensor-parallel over heads. Core c owns q-heads {2c, 2c+1} and kv-head
c//2. Wq/Wk/Wv column-sliced, Wo row-sliced on host; each core computes a full
o_proj partial [2048, 2048] (fp16) and the host sums the 8 partials.

v2 restructure (vs the 354us baseline):
  - S never goes to SBUF: softmax reads the PSUM S blocks directly.
    Per 128-row chunk p: S block J matmuls into a rotating [128,512] PSUM
    bank; per-block -max via tensor_reduce(max, negate=True); nm = min of
    the negated maxes; per-block Exp (ACT) reads PSUM with bias=nm and
    accum_out slots summed on DVE.
  - causal diagonal mask applied by the PE itself: one extra accumulating
    matmul S += maskA^T @ maskB_slice(p) (rank-128 ramp: adds
    -BIG*max(0, min(c-128p,128)-r)), so no vector-engine masking pass.
  - phase 1 (projections+LN) and phase 2/3 (attention+o_proj) are
    interleaved at emission: a unified work queue drains ph1 tiles /
    stage_b units / o_proj units into the stage_a dependency gaps so the
    PE never idles waiting on the softmax chain.
  - engine rebalance: LN-apply on ACT (Identity with per-partition
    scale/bias), rstd = Exp(-0.5*Ln(var+eps)) so the single ACT table
    (ln+exp+copy+identity) is loaded once; bn stats + qT/kT copies on DVE;
    vv/hoT/ob/pts copies split ACT/DVE; P normalization on GpSimd.
  - weight DMAs (wq chunks, wo, consts) issued on the GpSimd queue,
    activations (xr) and output stores on the Sync queue, so big weight
    loads don't serialize the x-tile stream.
  - fp32r for q/k path (full PE rate at free>=256), fp16 after softmax.
"""

from collections import deque
from contextlib import ExitStack

import numpy as np

import concourse.bass as bass
import concourse.bacc as bacc
import concourse.tile as tile
from concourse import mybir
from concourse import bass_utils

P = 128
T = 2048
C = 2048
NT = T // P       # 16 t-tiles
SB = 512          # superblock width
NSB = T // SB     # 4 superblocks
F32 = mybir.dt.float32
F32R = mybir.dt.float32r
F16 = mybir.dt.float16
AF = mybir.ActivationFunctionType
ALU = mybir.AluOpType
AX = mybir.AxisListType
BIG = 1.0e9

N_CORES = 8

# knobs
DRAIN_PER_P = 3     # work units drained between stage_a p-chunks
DRAIN_TAIL = 1      # extra units after each stage_a
NORM_ON_GPSIMD = False


def _build():
    nc = bacc.Bacc("TRN2", target_bir_lowering=False, debug=False,
                   num_devices=N_CORES)
    xt_d = nc.dram_tensor("xt", [T, C], F32, kind="ExternalInput").ap()
    wqkv_d = nc.dram_tensor("wqkv", [P, NT * 512], F32, kind="ExternalInput").ap()
    wo_d = nc.dram_tensor("wo", [P, 2 * C], F16, kind="ExternalInput").ap()
    fk_d = nc.dram_tensor("fk", [P, 1], F32, kind="ExternalInput").ap()
    ma_d = nc.dram_tensor("ma", [P, P], F32, kind="ExternalInput").ap()
    mb_d = nc.dram_tensor("mb", [P, 1024], F32, kind="ExternalInput").ap()
    id16_d = nc.dram_tensor("id16", [P, P], F16, kind="ExternalInput").ap()
    id32_d = nc.dram_tensor("id32", [P, P], F32, kind="ExternalInput").ap()
    out_d = nc.dram_tensor("out", [T, C], F16, kind="ExternalOutput").ap()

    with tile.TileContext(nc) as tc, ExitStack() as ctx:
        const = ctx.enter_context(tc.tile_pool(name="const", bufs=1))
        persist = ctx.enter_context(tc.tile_pool(name="persist", bufs=1))
        wqp = ctx.enter_context(tc.tile_pool(name="wqp", bufs=1))
        xrow_p = ctx.enter_context(tc.tile_pool(name="xrow", bufs=5))
        qln_p = ctx.enter_context(tc.tile_pool(name="qln", bufs=5))
        qsb_p = ctx.enter_context(tc.tile_pool(name="qsb", bufs=3))
        st1 = ctx.enter_context(tc.tile_pool(name="st1", bufs=3))
        st2 = ctx.enter_context(tc.tile_pool(name="st2", bufs=6))
        pe_p = ctx.enter_context(tc.tile_pool(name="peP", bufs=2))
        pb_p = ctx.enter_context(tc.tile_pool(name="pbP", bufs=2))
        pts_p = ctx.enter_context(tc.tile_pool(name="pts", bufs=3))
        ob_p = ctx.enter_context(tc.tile_pool(name="ob", bufs=4))
        psQ = ctx.enter_context(tc.tile_pool(name="psQ", bufs=2, space="PSUM"))
        ps512 = ctx.enter_context(tc.tile_pool(name="ps512", bufs=4, space="PSUM"))
        psPT = ctx.enter_context(tc.tile_pool(name="psPT", bufs=1, space="PSUM"))
        psO = ctx.enter_context(tc.tile_pool(name="psO", bufs=1, space="PSUM"))

        # ---------------- persistent SBUF state ----------------
        qT = persist.tile([P, 2 * T], F32R, tag="qT")   # [d, t] per q-head
        kT = persist.tile([P, T], F32R, tag="kT")       # [d, s] (fk folded)
        vv = persist.tile([P, T], F16, tag="vv")        # s-tile j at cols j*128
        hoT = persist.tile([P, 2 * T], F16, tag="hoT")  # [d, t] per head
        wo = persist.tile([P, 2 * C], F16, tag="wo")
        wq = wqp.tile([P, NT * 512], F32R, tag="wq")

        # x tile 0 on the scalar queue (ph1 tile 0 consumes it first)
        xr0 = xrow_p.tile([P, C], F32R, tag="xr")
        for c4 in range(4):
            nc.scalar.dma_start(xr0[:, c4 * 512:(c4 + 1) * 512],
                                xt_d[0:P, c4 * 512:(c4 + 1) * 512].bitcast(F32R))

        # weight loads share the sync queue, interleaved with the first
        # xr prefetches so neither stream starves
        for k in range(4):
            nc.sync.dma_start(wq[:, k * 512:(k + 1) * 512],
                              wqkv_d[:, k * 512:(k + 1) * 512].bitcast(F32R))
        id32 = const.tile([P, P], F32R, tag="id32")
        nc.sync.dma_start(id32[:], id32_d.bitcast(F32R))
        fk = const.tile([P, 1], F32, tag="fk")
        nc.sync.dma_start(fk[:], fk_d)
        id16 = const.tile([P, P], F16, tag="id16")
        nc.sync.dma_start(id16[:], id16_d)
        ma = const.tile([P, P], F32R, tag="ma")
        nc.sync.dma_start(ma[:], ma_d.bitcast(F32R))
        mb = const.tile([P, 1024], F32R, tag="mb")
        nc.sync.dma_start(mb[:], mb_d.bitcast(F32R))
        epsb = const.tile([P, 1], F32, tag="epsb")
        nc.vector.memset(epsb[:], 1e-5)
        nc.sync.dma_start(wo[:], wo_d)

        norm_eng = nc.gpsimd if NORM_ON_GPSIMD else nc.vector

        # xr prefetch: issue x-tile DMAs a few tiles ahead of consumption
        xr_pref = {0: xr0}
        pref_state = {"next": 1}

        def prefetch_xr(upto):
            while pref_state["next"] < min(upto, NT):
                i = pref_state["next"]
                xr = xrow_p.tile([P, C], F32R, tag="xr", name="xr")
                eng = nc.scalar if i <= 5 else nc.sync
                eng.dma_start(xr[:],
                              xt_d[i * P:(i + 1) * P, :].bitcast(F32R))
                xr_pref[i] = xr
                pref_state["next"] = i + 1

        for _g in range(1, 4):
            prefetch_xr(_g + 1)
            for k in range(4 * _g, 4 * _g + 4):
                nc.sync.dma_start(wq[:, k * 512:(k + 1) * 512],
                                  wqkv_d[:, k * 512:(k + 1) * 512].bitcast(F32R))

        # ---------------- phase 1: one t-tile, split into A/B units -------
        # A: xr DMA + 16 proj matmuls + bn stats (+ grouped Newton rstd on
        #    GpSimd for the pair of tiles) + qln (DVE, f32r) + vv copy (ACT)
        # B (drained later so the PE never waits on the LN chain): 3
        #    transposes (f32r) + qT single-CAST copy + kT scaled copy
        grp = {}

        def emit_newton(mvs):
            # rstd = 1/sqrt(var+eps): vpe+recip on DVE, seed+3 Newton steps
            # on GpSimd (SBUF-only smalls, keeps DVE/ACT free)
            n = len(mvs)
            vpe = st1.tile([P, 9], F32, tag="vpe")
            rstd = st1.tile([P, 9], F32, tag="rstd")
            for g, mv in enumerate(mvs):
                nc.vector.tensor_scalar_add(vpe[:, 3 * g:3 * g + 3],
                                            mv[:, 1::2], 1e-5)
            v = vpe[:, 0:3 * n]
            r = rstd[:, 0:3 * n]
            nc.vector.reciprocal(r, v)
            nc.vector.tensor_scalar(r, r, 1.0, 0.5, ALU.add, ALU.mult)
            for _ in range(3):
                ysq = st1.tile([P, 9], F32, tag="ysq")
                nc.vector.tensor_tensor(ysq[:, 0:3 * n], r, r, ALU.mult)
                nc.vector.tensor_tensor(ysq[:, 0:3 * n], v, ysq[:, 0:3 * n],
                                        ALU.mult)
                nc.vector.tensor_scalar(ysq[:, 0:3 * n], ysq[:, 0:3 * n],
                                        -0.5, 1.5, ALU.mult, ALU.add)
                nc.vector.tensor_tensor(r, r, ysq[:, 0:3 * n], ALU.mult)
            return rstd

        def emit_ph1_tile(i):
            prefetch_xr(i + 4)
            xr = xr_pref.pop(i)
            qkv = psQ.tile([P, 512], F32, tag="qkv")
            for k in range(NT):
                nc.tensor.matmul(qkv[:], xr[:, k * P:(k + 1) * P],
                                 wq[:, k * 512:(k + 1) * 512],
                                 start=(k == 0), stop=(k == NT - 1))
            # stage to SBUF once; everything downstream reads SBUF and the
            # PSUM slot frees immediately
            qsb = qsb_p.tile([P, 512], F32, tag="qsb")
            nc.scalar.copy(qsb[:, 0:256], qkv[:, 0:256])
            nc.vector.tensor_copy(qsb[:, 256:384], qkv[:, 256:384])
            nc.scalar.copy(vv[:, i * P:(i + 1) * P], qkv[:, 384:512])
            bnb = st1.tile([P, 18], F32, tag="bnb")
            mv = st1.tile([P, 6], F32, tag="mv")
            for j in range(3):
                nc.vector.bn_stats(bnb[:, 6 * j:6 * j + 6],
                                   qsb[:, j * P:(j + 1) * P])
                nc.vector.bn_aggr(mv[:, 2 * j:2 * j + 2], bnb[:, 6 * j:6 * j + 6])
            grp[i] = (qsb, mv)
            # finish LN for the previous pair once one further tile has
            # been emitted (gives the chain a tile of PE runway)
            ids = [t for t in (i - 2, i - 1) if t in grp]
            if not ids:
                return
            if i == NT - 1:
                ids.append(i)  # last tile: no further runway available
            rstd = emit_newton([grp[t][1] for t in ids])
            for g, t in enumerate(ids):
                qsb_t, mv_t = grp.pop(t)
                qln = qln_p.tile([P, 384], F32R, tag="qln")
                for j in range(3):
                    nc.vector.tensor_scalar(
                        qln[:, j * P:(j + 1) * P],
                        qsb_t[:, j * P:(j + 1) * P],
                        mv_t[:, 2 * j:2 * j + 1],
                        rstd[:, 3 * g + j:3 * g + j + 1],
                        ALU.subtract, ALU.mult)
                bwork.append(("tB", counters["tick"], make_b_tile_unit(t, qln)))
                counters["queued"] += 1

        def make_b_tile_unit(i, qln):
            def run():
                counters["tilesB"] += 1
                pt = ps512.tile([P, 512], F32, tag="ps", name="pt")
                for j in range(3):
                    nc.tensor.transpose(pt[:, j * P:(j + 1) * P].bitcast(F32R),
                                        qln[:, j * P:(j + 1) * P], id32[:])
                # qT layout: cols = i*256 + h*128 -> both heads in one CAST
                nc.scalar.copy(qT[:, i * 256:(i + 1) * 256],
                               pt[:, 0:256].bitcast(F32R))
                nc.scalar.mul(kT[:, i * P:(i + 1) * P],
                              pt[:, 2 * P:3 * P].bitcast(F32R), fk[:])
            return run

        # ---------------- work queue ----------------
        tile_q = deque(range(NT))
        bwork = deque()
        counters = {"queued": 0, "drained": 0, "tilesB": 0, "tick": 0}
        pair_cum = {}

        def tile_q_has(i):
            return i in tile_q

        def emit_tiles_upto(n):
            while tile_q and tile_q[0] < n:
                emit_ph1_tile(tile_q.popleft())
                # give this tile's LN chain runway: pull older pending work
                # (b/o units or older tiles' B-units) in between
                counters["tick"] += 1
                k = 2
                while k > 0 and bwork:
                    kind, tick, fn = bwork[0]
                    if kind == "tB" and counters["tick"] - tick < 2:
                        break
                    run_unit(bwork.popleft())
                    k -= 1

        def run_unit(u):
            u[2]()
            counters["drained"] += 1
            counters["tick"] += 1

        def drain(n, force=False):
            while n > 0:
                if bwork:
                    kind, tick, fn = bwork[0]
                    # a fresh tile-B unit would stall the PE on its LN chain:
                    # prefer other work until it has aged 2 ticks
                    if (kind == "tB" and not force and tile_q
                            and counters["tick"] - tick < 2):
                        emit_ph1_tile(tile_q.popleft())
                        counters["tick"] += 1
                    else:
                        run_unit(bwork.popleft())
                elif tile_q:
                    emit_ph1_tile(tile_q.popleft())
                    counters["tick"] += 1
                else:
                    return
                n -= 1

        # ---------------- phase 2: stage_a (S + softmax) ----------------
        def emit_stage_a(I, h):
            # Pe/Pb pools have bufs=2: before reusing the slot of pair n-2,
            # every deferred unit reading that pair's Pb must be emitted.
            pidx = 2 * I + h
            need = pair_cum.get(pidx - 2, 0)
            while counters["drained"] < need and bwork:
                run_unit(bwork.popleft())
            # the S matmuls read qT/kT tiles <= 4I+3, written by deferred
            # tile-B units: those must be EMITTED before we emit the reads
            while counters["tilesB"] < 4 * (I + 1) and (bwork or tile_q):
                drain(1)
            L = (I + 1) * SB
            Pe = pe_p.tile([P, 4 * T], F16, tag="Pe")
            Pb = pb_p.tile([P, 4 * T], F16, tag="Pb")
            Z = st2.tile([P, 4], F32, tag="Z", bufs=2)
            for p in range(4):
                it = I * 4 + p
                lq = qT[:, it * 256 + h * P:it * 256 + (h + 1) * P]
                rmb = st2.tile([P, 4], F32, tag="rmb")
                sblocks = []
                for J in range(I + 1):
                    sp = ps512.tile([P, 512], F32, tag="ps")
                    nc.tensor.matmul(sp[:], lq, kT[:, J * SB:(J + 1) * SB],
                                     start=True, stop=(J != I))
                    if J == I:
                        nc.tensor.matmul(
                            sp[:], ma[:],
                            mb[:, 512 - 128 * p:1024 - 128 * p],
                            start=False, stop=True)
                    nc.vector.tensor_reduce(rmb[:, J:J + 1], sp[:], AX.X,
                                            ALU.max, negate=True)
                    sblocks.append(sp)
                if I == 0:
                    nm = rmb[:, 0:1]
                else:
                    nmt = st2.tile([P, 1], F32, tag="nmt")
                    nc.vector.tensor_reduce(nmt[:], rmb[:, 0:I + 1], AX.X,
                                            ALU.min)
                    nm = nmt[:]
                Zp = st2.tile([P, 4], F32, tag="Zp")
                for J in range(I + 1):
                    nc.scalar.activation(
                        Pe[:, p * T + J * SB:p * T + (J + 1) * SB],
                        sblocks[J][:], AF.Exp, bias=nm,
                        accum_out=(Z[:, p:p + 1] if I == 0
                                   else Zp[:, J:J + 1]))
                if I >= 1:
                    nc.vector.tensor_reduce(Z[:, p:p + 1], Zp[:, 0:I + 1],
                                            AX.X, ALU.add)
                drain(DRAIN_PER_P)
            Zi = st2.tile([P, 4], F32, tag="Zi", bufs=2)
            nc.vector.reciprocal(Zi[:], Z[:])
            for p in range(4):
                norm_eng.tensor_scalar_mul(Pb[:, p * T:p * T + L],
                                           Pe[:, p * T:p * T + L],
                                           Zi[:, p:p + 1])
            drain(DRAIN_TAIL)
            return Pb

        # ---------------- stage_b + o_proj units ----------------
        def make_b_units(I, h, Pb):
            nst = 4 * (I + 1)
            state = {}

            def unit(j):
                def run():
                    if j == 0:
                        state["oT"] = psO.tile([P, SB], F32, tag="oT", name="oT")
                    oT = state["oT"]
                    ptpf = psPT.tile([P, 2 * SB], F16, tag="ptp", name="ptpf")
                    half = (j % 2) * SB
                    ptp = ptpf[:, half:half + SB]
                    for p in range(4):
                        nc.tensor.transpose(
                            ptp[:, p * P:(p + 1) * P],
                            Pb[:, p * T + j * P:p * T + (j + 1) * P],
                            id16[:])
                    pts = pts_p.tile([P, SB], F16, tag="pts")
                    if j % 2 == 0:
                        nc.scalar.copy(pts[:], ptp[:])
                    else:
                        nc.vector.tensor_copy(pts[:], ptp[:])
                    nc.tensor.matmul(oT[:], vv[:, j * P:(j + 1) * P], pts[:],
                                     start=(j == 0), stop=(j == nst - 1))
                    if j == nst - 1:
                        nc.scalar.copy(hoT[:, h * T + I * SB:h * T + (I + 1) * SB],
                                       oT[:])
                return run
            return [unit(j) for j in range(nst)]

        def make_oproj_units(I):
            units = []
            for it in range(4 * I, 4 * I + 4):
                for e in range(4):
                    def run(it=it, e=e):
                        po = ps512.tile([P, 512], F32, tag="ps")
                        for hh in range(2):
                            nc.tensor.matmul(
                                po[:],
                                hoT[:, hh * T + it * P:hh * T + (it + 1) * P],
                                wo[:, hh * C + e * SB:hh * C + (e + 1) * SB],
                                start=(hh == 0), stop=(hh == 1))
                        ob = ob_p.tile([P, SB], F16, tag="ob")
                        nc.scalar.copy(ob[:], po[:])
                        nc.sync.dma_start(
                            out_d[it * P:(it + 1) * P, e * SB:(e + 1) * SB],
                            ob[:])
                    units.append(run)
            return units

        # ---------------- schedule ----------------
        for g in range(NSB):
            emit_tiles_upto(4 * (g + 1))
            for h in range(2):
                Pb = emit_stage_a(g, h)
                bu = make_b_units(g, h, Pb)
                bwork.extend(("b", counters["tick"], u) for u in bu)
                counters["queued"] += len(bu)
                if h == 1:
                    ou = make_oproj_units(g)
                    bwork.extend(("o", counters["tick"], u) for u in ou)
                    counters["queued"] += len(ou)
                pair_cum[2 * g + h] = counters["queued"]
        while tile_q:
            emit_ph1_tile(tile_q.popleft())
        if grp:
            ids = sorted(grp)
            rstd = emit_newton([grp[t][1] for t in ids])
            for g, t in enumerate(ids):
                qsb_t, mv_t = grp.pop(t)
                qln = qln_p.tile([P, 384], F32R, tag="qln", name="qln")
                for j in range(3):
                    nc.vector.tensor_scalar(
                        qln[:, j * P:(j + 1) * P],
                        qsb_t[:, j * P:(j + 1) * P],
                        mv_t[:, 2 * j:2 * j + 1],
                        rstd[:, 3 * g + j:3 * g + j + 1],
                        ALU.subtract, ALU.mult)
                bwork.append(("tB", counters["tick"], make_b_tile_unit(t, qln)))
                counters["queued"] += 1
        while bwork:
            run_unit(bwork.popleft())

    nc.compile()
    return nc


def _host_inputs(x, Wq, Wk, Wv, Wo, gq, gk, temp):
    """Build the 8 per-core input maps (host-side shard + layout prep)."""
    x2 = np.ascontiguousarray(np.asarray(x, dtype=np.float32).reshape(T, C))
    # xt[i*128 + p, k*128 + u] = x[128*i + u, 128*k + p]
    xt = np.ascontiguousarray(
        x2.reshape(NT, P, NT, P).transpose(0, 3, 2, 1).reshape(T, C))
    scale = np.float32(min(np.exp(np.float32(temp)), np.float32(50.0)))
    fk = np.ascontiguousarray(
        (np.asarray(gq, np.float32) * np.asarray(gk, np.float32)
         * scale).reshape(P, 1))
    id16 = np.eye(P, dtype=np.float16)
    id32 = np.eye(P, dtype=np.float32)
    # causal ramp mask factors: (ma^T@mb_slice)(r,c) = -BIG*max(0,min(c',128)-r)
    r = np.arange(P)
    ma = (r[None, :] <= r[:, None]).astype(np.float32)   # ma[i, r] = r <= i
    j = np.arange(1024)
    mb = np.where(j[None, :] - 512 >= (r + 1)[:, None], -BIG,
                  0.0).astype(np.float32)
    in_maps = []
    for core in range(N_CORES):
        q0 = core * 256
        kv0 = (core // 2) * P
        wqkv = np.concatenate([Wq[:, q0:q0 + 256],
                               Wk[:, kv0:kv0 + P],
                               Wv[:, kv0:kv0 + P]], axis=1).astype(np.float32)
        # wqkv_t[p, k*512 + j] = wqkv[128k + p, j]
        wqkv_t = np.ascontiguousarray(
            wqkv.reshape(NT, P, 512).transpose(1, 0, 2).reshape(P, NT * 512))
        # wo_t[p, h*2048 + c] = Wo[q0 + 128h + p, c], fp16
        wo_t = np.ascontiguousarray(
            np.asarray(Wo[q0:q0 + 256, :], np.float32)
            .reshape(2, P, C).transpose(1, 0, 2).reshape(P, 2 * C)
            .astype(np.float16))
        in_maps.append({
            "xt": xt,
            "wqkv": wqkv_t,
            "wo": wo_t,
            "fk": fk,
            "ma": ma,
            "mb": mb,
            "id16": id16,
            "id32": id32,
        })
    return in_maps


_NC_CACHE = {}


def _get_nc():
    if "nc" not in _NC_CACHE:
        _NC_CACHE["nc"] = _build()
    return _NC_CACHE["nc"]


def run(inputs, trace=False):
    nc = _get_nc()
    in_maps = _host_inputs(**inputs)
    res = bass_utils.run_bass_kernel_spmd(
        nc, in_maps, core_ids=list(range(N_CORES)), trace=trace)
    acc = res.results[0]["out"].astype(np.float32)
    for corer in res.results[1:]:
        acc = acc + corer["out"].astype(np.float32)
    return acc.reshape(1, T, C), res


def kernel(**inputs):
    out, _ = run(inputs, trace=False)
    return out


# revision 22
# speedup vs baseline: 1.0530x; 1.0530x over previous
"""GroupedQueryAttention (B=1, T=2048, C=2048, H=16, KVH=4, D=128) on 8 trn2 cores.

Sharding: tensor-parallel over heads. Core c owns q-heads {2c, 2c+1} and kv-head
c//2. Wq/Wk/Wv column-sliced, Wo row-sliced on host; each core computes a full
o_proj partial [2048, 2048] (fp16) and the host sums the 8 partials.

v2 restructure (vs the 354us baseline):
  - S never goes to SBUF: softmax reads the PSUM S blocks directly.
    Per 128-row chunk p: S block J matmuls into a rotating [128,512] PSUM
    bank; per-block -max via tensor_reduce(max, negate=True); nm = min of
    the negated maxes; per-block Exp (ACT) reads PSUM with bias=nm and
    accum_out slots summed on DVE.
  - causal diagonal mask applied by the PE itself: one extra accumulating
    matmul S += maskA^T @ maskB_slice(p) (rank-128 ramp: adds
    -BIG*max(0, min(c-128p,128)-r)), so no vector-engine masking pass.
  - phase 1 (projections+LN) and phase 2/3 (attention+o_proj) are
    interleaved at emission: a unified work queue drains ph1 tiles /
    stage_b units / o_proj units into the stage_a dependency gaps so the
    PE never idles waiting on the softmax chain.
  - engine rebalance: LN-apply on ACT (Identity with per-partition
    scale/bias), rstd = Exp(-0.5*Ln(var+eps)) so the single ACT table
    (ln+exp+copy+identity) is loaded once; bn stats + qT/kT copies on DVE;
    vv/hoT/ob/pts copies split ACT/DVE; P normalization on GpSimd.
  - weight DMAs (wq chunks, wo, consts) issued on the GpSimd queue,
    activations (xr) and output stores on the Sync queue, so big weight
    loads don't serialize the x-tile stream.
  - fp32r for q/k path (full PE rate at free>=256), fp16 after softmax.
"""

from collections import deque
from contextlib import ExitStack

import numpy as np

import concourse.bass as bass
import concourse.bacc as bacc
import concourse.tile as tile
from concourse import mybir
from concourse import bass_utils

P = 128
T = 2048
C = 2048
NT = T // P       # 16 t-tiles
SB = 512          # superblock width
NSB = T // SB     # 4 superblocks
F32 = mybir.dt.float32
F32R = mybir.dt.float32r
F16 = mybir.dt.float16
AF = mybir.ActivationFunctionType
ALU = mybir.AluOpType
AX = mybir.AxisListType
BIG = 1.0e9

N_CORES = 8

# knobs
DRAIN_PER_P = 3     # work units drained between stage_a p-chunks
DRAIN_TAIL = 1      # extra units after each stage_a
NORM_ON_GPSIMD = False


def _build():
    nc = bacc.Bacc("TRN2", target_bir_lowering=False, debug=False,
                   num_devices=N_CORES)
    xt_d = nc.dram_tensor("xt", [T, C], F32, kind="ExternalInput").ap()
    wqkv_d = nc.dram_tensor("wqkv", [P, NT * 512], F32, kind="ExternalInput").ap()
    wo_d = nc.dram_tensor("wo", [P, 2 * C], F16, kind="ExternalInput").ap()
    fk_d = nc.dram_tensor("fk", [P, 1], F32, kind="ExternalInput").ap()
    ma_d = nc.dram_tensor("ma", [P, P], F32, kind="ExternalInput").ap()
    mb_d = nc.dram_tensor("mb", [P, 1024], F32, kind="ExternalInput").ap()
    id16_d = nc.dram_tensor("id16", [P, P], F16, kind="ExternalInput").ap()
    id32_d = nc.dram_tensor("id32", [P, P], F32, kind="ExternalInput").ap()
    out_d = nc.dram_tensor("out", [T, C], F16, kind="ExternalOutput").ap()

    with tile.TileContext(nc) as tc, ExitStack() as ctx:
        const = ctx.enter_context(tc.tile_pool(name="const", bufs=1))
        persist = ctx.enter_context(tc.tile_pool(name="persist", bufs=1))
        wqp = ctx.enter_context(tc.tile_pool(name="wqp", bufs=1))
        xrow_p = ctx.enter_context(tc.tile_pool(name="xrow", bufs=5))
        qln_p = ctx.enter_context(tc.tile_pool(name="qln", bufs=5))
        qsb_p = ctx.enter_context(tc.tile_pool(name="qsb", bufs=3))
        st1 = ctx.enter_context(tc.tile_pool(name="st1", bufs=3))
        st2 = ctx.enter_context(tc.tile_pool(name="st2", bufs=6))
        pe_p = ctx.enter_context(tc.tile_pool(name="peP", bufs=2))
        pb_p = ctx.enter_context(tc.tile_pool(name="pbP", bufs=2))
        pts_p = ctx.enter_context(tc.tile_pool(name="pts", bufs=3))
        ob_p = ctx.enter_context(tc.tile_pool(name="ob", bufs=4))
        psQ = ctx.enter_context(tc.tile_pool(name="psQ", bufs=2, space="PSUM"))
        ps512 = ctx.enter_context(tc.tile_pool(name="ps512", bufs=4, space="PSUM"))
        psPT = ctx.enter_context(tc.tile_pool(name="psPT", bufs=1, space="PSUM"))
        psO = ctx.enter_context(tc.tile_pool(name="psO", bufs=1, space="PSUM"))

        # ---------------- persistent SBUF state ----------------
        qT = persist.tile([P, 2 * T], F32R, tag="qT")   # [d, t] per q-head
        kT = persist.tile([P, T], F32R, tag="kT")       # [d, s] (fk folded)
        vv = persist.tile([P, T], F16, tag="vv")        # s-tile j at cols j*128
        hoT = persist.tile([P, 2 * T], F16, tag="hoT")  # [d, t] per head
        wo = persist.tile([P, 2 * C], F16, tag="wo")
        wq = wqp.tile([P, NT * 512], F32R, tag="wq")

        # x tile 0 on the scalar queue (ph1 tile 0 consumes it first)
        xr0 = xrow_p.tile([P, C], F32R, tag="xr")
        for c4 in range(4):
            nc.scalar.dma_start(xr0[:, c4 * 512:(c4 + 1) * 512],
                                xt_d[0:P, c4 * 512:(c4 + 1) * 512].bitcast(F32R))

        # weight loads share the sync queue, interleaved with the first
        # xr prefetches so neither stream starves
        for k in range(4):
            nc.sync.dma_start(wq[:, k * 512:(k + 1) * 512],
                              wqkv_d[:, k * 512:(k + 1) * 512].bitcast(F32R))
        id32 = const.tile([P, P], F32R, tag="id32")
        nc.sync.dma_start(id32[:], id32_d.bitcast(F32R))
        fk = const.tile([P, 1], F32, tag="fk")
        nc.sync.dma_start(fk[:], fk_d)
        id16 = const.tile([P, P], F16, tag="id16")
        nc.sync.dma_start(id16[:], id16_d)
        ma = const.tile([P, P], F32R, tag="ma")
        nc.sync.dma_start(ma[:], ma_d.bitcast(F32R))
        mb = const.tile([P, 1024], F32R, tag="mb")
        nc.sync.dma_start(mb[:], mb_d.bitcast(F32R))
        epsb = const.tile([P, 1], F32, tag="epsb")
        nc.vector.memset(epsb[:], 1e-5)
        nc.sync.dma_start(wo[:], wo_d)

        norm_eng = nc.gpsimd if NORM_ON_GPSIMD else nc.vector

        # xr prefetch: issue x-tile DMAs a few tiles ahead of consumption
        xr_pref = {0: xr0}
        pref_state = {"next": 1}

        def prefetch_xr(upto):
            while pref_state["next"] < min(upto, NT):
                i = pref_state["next"]
                xr = xrow_p.tile([P, C], F32R, tag="xr", name="xr")
                eng = nc.scalar if i <= 5 else nc.sync
                eng.dma_start(xr[:],
                              xt_d[i * P:(i + 1) * P, :].bitcast(F32R))
                xr_pref[i] = xr
                pref_state["next"] = i + 1

        for _g in range(1, 4):
            prefetch_xr(_g + 1)
            for k in range(4 * _g, 4 * _g + 4):
                nc.sync.dma_start(wq[:, k * 512:(k + 1) * 512],
                                  wqkv_d[:, k * 512:(k + 1) * 512].bitcast(F32R))

        # ---------------- phase 1: one t-tile, split into A/B units -------
        # A: xr DMA + 16 proj matmuls + bn stats (+ grouped Newton rstd on
        #    GpSimd for the pair of tiles) + qln (DVE, f32r) + vv copy (ACT)
        # B (drained later so the PE never waits on the LN chain): 3
        #    transposes (f32r) + qT single-CAST copy + kT scaled copy
        grp = {}

        def emit_newton(mvs):
            # rstd = 1/sqrt(var+eps): vpe+recip on DVE, seed+3 Newton steps
            # on GpSimd (SBUF-only smalls, keeps DVE/ACT free)
            n = len(mvs)
            vpe = st1.tile([P, 9], F32, tag="vpe")
            rstd = st1.tile([P, 9], F32, tag="rstd")
            for g, mv in enumerate(mvs):
                nc.vector.tensor_scalar_add(vpe[:, 3 * g:3 * g + 3],
                                            mv[:, 1::2], 1e-5)
            v = vpe[:, 0:3 * n]
            r = rstd[:, 0:3 * n]
            nc.vector.reciprocal(r, v)
            nc.vector.tensor_scalar(r, r, 1.0, 0.5, ALU.add, ALU.mult)
            for _ in range(3):
                ysq = st1.tile([P, 9], F32, tag="ysq")
                nc.vector.tensor_tensor(ysq[:, 0:3 * n], r, r, ALU.mult)
                nc.vector.tensor_tensor(ysq[:, 0:3 * n], v, ysq[:, 0:3 * n],
                                        ALU.mult)
                nc.vector.tensor_scalar(ysq[:, 0:3 * n], ysq[:, 0:3 * n],
                                        -0.5, 1.5, ALU.mult, ALU.add)
                nc.vector.tensor_tensor(r, r, ysq[:, 0:3 * n], ALU.mult)
            return rstd

        def emit_ph1_tile(i):
            prefetch_xr(i + 4)
            xr = xr_pref.pop(i)
            qkv = psQ.tile([P, 512], F32, tag="qkv")
            for k in range(NT):
                nc.tensor.matmul(qkv[:], xr[:, k * P:(k + 1) * P],
                                 wq[:, k * 512:(k + 1) * 512],
                                 start=(k == 0), stop=(k == NT - 1))
            # stage to SBUF once; everything downstream reads SBUF and the
            # PSUM slot frees immediately
            qsb = qsb_p.tile([P, 512], F32, tag="qsb")
            nc.scalar.copy(qsb[:, 0:256], qkv[:, 0:256])
            nc.vector.tensor_copy(qsb[:, 256:384], qkv[:, 256:384])
            nc.scalar.copy(vv[:, i * P:(i + 1) * P], qkv[:, 384:512])
            bnb = st1.tile([P, 18], F32, tag="bnb")
            mv = st1.tile([P, 6], F32, tag="mv")
            for j in range(3):
                nc.vector.bn_stats(bnb[:, 6 * j:6 * j + 6],
                                   qsb[:, j * P:(j + 1) * P])
                nc.vector.bn_aggr(mv[:, 2 * j:2 * j + 2], bnb[:, 6 * j:6 * j + 6])
            grp[i] = (qsb, mv)
            # finish LN for the previous pair once one further tile has
            # been emitted (gives the chain a tile of PE runway)
            ids = [t for t in (i - 2, i - 1) if t in grp]
            if not ids:
                return
            if i == NT - 1:
                ids.append(i)  # last tile: no further runway available
            rstd = emit_newton([grp[t][1] for t in ids])
            for g, t in enumerate(ids):
                qsb_t, mv_t = grp.pop(t)
                qln = qln_p.tile([P, 384], F32R, tag="qln")
                for j in range(3):
                    nc.vector.tensor_scalar(
                        qln[:, j * P:(j + 1) * P],
                        qsb_t[:, j * P:(j + 1) * P],
                        mv_t[:, 2 * j:2 * j + 1],
                        rstd[:, 3 * g + j:3 * g + j + 1],
                        ALU.subtract, ALU.mult)
                bwork.append(("tB", counters["tick"], make_b_tile_unit(t, qln)))
                counters["queued"] += 1

        def make_b_tile_unit(i, qln):
            def run():
                counters["tilesB"] += 1
                pt = ps512.tile([P, 512], F32, tag="ps", name="pt")
                for j in range(3):
                    nc.tensor.transpose(pt[:, j * P:(j + 1) * P].bitcast(F32R),
                                        qln[:, j * P:(j + 1) * P], id32[:])
                # qT layout: cols = i*256 + h*128 -> both heads in one CAST
                nc.scalar.copy(qT[:, i * 256:(i + 1) * 256],
                               pt[:, 0:256].bitcast(F32R))
                nc.scalar.mul(kT[:, i * P:(i + 1) * P],
                              pt[:, 2 * P:3 * P].bitcast(F32R), fk[:])
            return run

        # ---------------- work queue ----------------
        tile_q = deque(range(NT))
        bwork = deque()
        counters = {"queued": 0, "drained": 0, "tilesB": 0, "tick": 0}
        pair_cum = {}

        def tile_q_has(i):
            return i in tile_q

        def emit_tiles_upto(n):
            while tile_q and tile_q[0] < n:
                emit_ph1_tile(tile_q.popleft())
                # give this tile's LN chain runway: pull older pending work
                # (b/o units or older tiles' B-units) in between
                counters["tick"] += 1
                k = 2
                while k > 0 and bwork:
                    kind, tick, fn = bwork[0]
                    if kind == "tB" and counters["tick"] - tick < 2:
                        break
                    run_unit(bwork.popleft())
                    k -= 1

        def run_unit(u):
            u[2]()
            counters["drained"] += 1
            counters["tick"] += 1

        def drain(n, force=False):
            while n > 0:
                if bwork:
                    kind, tick, fn = bwork[0]
                    # a fresh tile-B unit would stall the PE on its LN chain:
                    # prefer other work until it has aged 2 ticks
                    if (kind == "tB" and not force and tile_q
                            and counters["tick"] - tick < 2):
                        emit_ph1_tile(tile_q.popleft())
                        counters["tick"] += 1
                    else:
                        run_unit(bwork.popleft())
                elif tile_q:
                    emit_ph1_tile(tile_q.popleft())
                    counters["tick"] += 1
                else:
                    return
                n -= 1

        # ---------------- phase 2: stage_a (S + softmax) ----------------
        def emit_stage_a(I, h, pidx):
            # Pe/Pb pools have bufs=2: before reusing the slot of pair n-2,
            # every deferred unit reading that pair's Pb must be emitted.
            need = pair_cum.get(pidx - 2, 0)
            while counters["drained"] < need and bwork:
                run_unit(bwork.popleft())
            # the S matmuls read qT/kT tiles <= 4I+3, written by deferred
            # tile-B units: those must be EMITTED before we emit the reads
            while counters["tilesB"] < 4 * (I + 1) and (bwork or tile_q):
                drain(1)
            L = (I + 1) * SB
            Pe = pe_p.tile([P, 4 * T], F16, tag="Pe")
            Pb = pb_p.tile([P, 4 * T], F16, tag="Pb")
            Z = st2.tile([P, 4], F32, tag="Z", bufs=2)
            for p in range(4):
                it = I * 4 + p
                lq = qT[:, it * 256 + h * P:it * 256 + (h + 1) * P]
                rmb = st2.tile([P, 4], F32, tag="rmb")
                sblocks = []
                for J in range(I + 1):
                    sp = ps512.tile([P, 512], F32, tag="ps")
                    nc.tensor.matmul(sp[:], lq, kT[:, J * SB:(J + 1) * SB],
                                     start=True, stop=(J != I))
                    if J == I:
                        nc.tensor.matmul(
                            sp[:], ma[:],
                            mb[:, 512 - 128 * p:1024 - 128 * p],
                            start=False, stop=True)
                    nc.vector.tensor_reduce(rmb[:, J:J + 1], sp[:], AX.X,
                                            ALU.max, negate=True)
                    sblocks.append(sp)
                if I == 0:
                    nm = rmb[:, 0:1]
                else:
                    nmt = st2.tile([P, 1], F32, tag="nmt")
                    nc.vector.tensor_reduce(nmt[:], rmb[:, 0:I + 1], AX.X,
                                            ALU.min)
                    nm = nmt[:]
                Zp = st2.tile([P, 4], F32, tag="Zp")
                for J in range(I + 1):
                    nc.scalar.activation(
                        Pe[:, p * T + J * SB:p * T + (J + 1) * SB],
                        sblocks[J][:], AF.Exp, bias=nm,
                        accum_out=(Z[:, p:p + 1] if I == 0
                                   else Zp[:, J:J + 1]))
                if I >= 1:
                    nc.vector.tensor_reduce(Z[:, p:p + 1], Zp[:, 0:I + 1],
                                            AX.X, ALU.add)
                drain(DRAIN_PER_P)
            Zi = st2.tile([P, 4], F32, tag="Zi", bufs=2)
            nc.vector.reciprocal(Zi[:], Z[:])
            for p in range(4):
                norm_eng.tensor_scalar_mul(Pb[:, p * T:p * T + L],
                                           Pe[:, p * T:p * T + L],
                                           Zi[:, p:p + 1])
            drain(DRAIN_TAIL)
            return Pb

        # ---------------- stage_b + o_proj units ----------------
        def make_b_units(I, h, Pb):
            nst = 4 * (I + 1)
            state = {}

            def unit(j):
                def run():
                    if j == 0:
                        state["oT"] = psO.tile([P, SB], F32, tag="oT", name="oT")
                    oT = state["oT"]
                    ptpf = psPT.tile([P, 2 * SB], F16, tag="ptp", name="ptpf")
                    half = (j % 2) * SB
                    ptp = ptpf[:, half:half + SB]
                    for p in range(4):
                        nc.tensor.transpose(
                            ptp[:, p * P:(p + 1) * P],
                            Pb[:, p * T + j * P:p * T + (j + 1) * P],
                            id16[:])
                    pts = pts_p.tile([P, SB], F16, tag="pts")
                    if j % 2 == 0:
                        nc.scalar.copy(pts[:], ptp[:])
                    else:
                        nc.vector.tensor_copy(pts[:], ptp[:])
                    nc.tensor.matmul(oT[:], vv[:, j * P:(j + 1) * P], pts[:],
                                     start=(j == 0), stop=(j == nst - 1))
                    if j == nst - 1:
                        nc.scalar.copy(hoT[:, h * T + I * SB:h * T + (I + 1) * SB],
                                       oT[:])
                return run
            return [unit(j) for j in range(nst)]

        def make_oproj_units(I):
            units = []
            for it in range(4 * I, 4 * I + 4):
                for e in range(4):
                    def run(it=it, e=e):
                        po = ps512.tile([P, 512], F32, tag="ps")
                        for hh in range(2):
                            nc.tensor.matmul(
                                po[:],
                                hoT[:, hh * T + it * P:hh * T + (it + 1) * P],
                                wo[:, hh * C + e * SB:hh * C + (e + 1) * SB],
                                start=(hh == 0), stop=(hh == 1))
                        ob = ob_p.tile([P, SB], F16, tag="ob")
                        nc.scalar.copy(ob[:], po[:])
                        nc.sync.dma_start(
                            out_d[it * P:(it + 1) * P, e * SB:(e + 1) * SB],
                            ob[:])
                    units.append(run)
            return units

        # ---------------- schedule ----------------
        # boustrophedon pair order: ascending h=0 then descending h=1, so
        # the final pair has the smallest stage_b tail
        pair_order = [(0, 0), (1, 0), (2, 0), (3, 0),
                      (3, 1), (2, 1), (1, 1), (0, 1)]
        done_h0 = set()
        for pidx, (g, h) in enumerate(pair_order):
            emit_tiles_upto(4 * (g + 1))
            Pb = emit_stage_a(g, h, pidx)
            bu = make_b_units(g, h, Pb)
            bwork.extend(("b", counters["tick"], u) for u in bu)
            counters["queued"] += len(bu)
            if h == 1:
                ou = make_oproj_units(g)
                bwork.extend(("o", counters["tick"], u) for u in ou)
                counters["queued"] += len(ou)
            pair_cum[pidx] = counters["queued"]
        while tile_q:
            emit_ph1_tile(tile_q.popleft())
        if grp:
            ids = sorted(grp)
            rstd = emit_newton([grp[t][1] for t in ids])
            for g, t in enumerate(ids):
                qsb_t, mv_t = grp.pop(t)
                qln = qln_p.tile([P, 384], F32R, tag="qln", name="qln")
                for j in range(3):
                    nc.vector.tensor_scalar(
                        qln[:, j * P:(j + 1) * P],
                        qsb_t[:, j * P:(j + 1) * P],
                        mv_t[:, 2 * j:2 * j + 1],
                        rstd[:, 3 * g + j:3 * g + j + 1],
                        ALU.subtract, ALU.mult)
                bwork.append(("tB", counters["tick"], make_b_tile_unit(t, qln)))
                counters["queued"] += 1
        while bwork:
            run_unit(bwork.popleft())

    nc.compile()
    return nc


def _host_inputs(x, Wq, Wk, Wv, Wo, gq, gk, temp):
    """Build the 8 per-core input maps (host-side shard + layout prep)."""
    x2 = np.ascontiguousarray(np.asarray(x, dtype=np.float32).reshape(T, C))
    # xt[i*128 + p, k*128 + u] = x[128*i + u, 128*k + p]
    xt = np.ascontiguousarray(
        x2.reshape(NT, P, NT, P).transpose(0, 3, 2, 1).reshape(T, C))
    scale = np.float32(min(np.exp(np.float32(temp)), np.float32(50.0)))
    fk = np.ascontiguousarray(
        (np.asarray(gq, np.float32) * np.asarray(gk, np.float32)
         * scale).reshape(P, 1))
    id16 = np.eye(P, dtype=np.float16)
    id32 = np.eye(P, dtype=np.float32)
    # causal ramp mask factors: (ma^T@mb_slice)(r,c) = -BIG*max(0,min(c',128)-r)
    r = np.arange(P)
    ma = (r[None, :] <= r[:, None]).astype(np.float32)   # ma[i, r] = r <= i
    j = np.arange(1024)
    mb = np.where(j[None, :] - 512 >= (r + 1)[:, None], -BIG,
                  0.0).astype(np.float32)
    in_maps = []
    for core in range(N_CORES):
        q0 = core * 256
        kv0 = (core // 2) * P
        wqkv = np.concatenate([Wq[:, q0:q0 + 256],
                               Wk[:, kv0:kv0 + P],
                               Wv[:, kv0:kv0 + P]], axis=1).astype(np.float32)
        # wqkv_t[p, k*512 + j] = wqkv[128k + p, j]
        wqkv_t = np.ascontiguousarray(
            wqkv.reshape(NT, P, 512).transpose(1, 0, 2).reshape(P, NT * 512))
        # wo_t[p, h*2048 + c] = Wo[q0 + 128h + p, c], fp16
        wo_t = np.ascontiguousarray(
            np.asarray(Wo[q0:q0 + 256, :], np.float32)
            .reshape(2, P, C).transpose(1, 0, 2).reshape(P, 2 * C)
            .astype(np.float16))
        in_maps.append({
            "xt": xt,
            "wqkv": wqkv_t,
            "wo": wo_t,
            "fk": fk,
            "ma": ma,
            "mb": mb,
            "id16": id16,
            "id32": id32,
        })
    return in_maps


_NC_CACHE = {}


def _get_nc():
    if "nc" not in _NC_CACHE:
        _NC_CACHE["nc"] = _build()
    return _NC_CACHE["nc"]


def run(inputs, trace=False):
    nc = _get_nc()
    in_maps = _host_inputs(**inputs)
    res = bass_utils.run_bass_kernel_spmd(
        nc, in_maps, core_ids=list(range(N_CORES)), trace=trace)
    acc = res.results[0]["out"].astype(np.float32)
    for corer in res.results[1:]:
        acc = acc + corer["out"].astype(np.float32)
    return acc.reshape(1, T, C), res


def kernel(**inputs):
    out, _ = run(inputs, trace=False)
    return out
